# revision 2
# baseline (speedup 1.0000x reference)
"""Trainium2 Bass kernel v2 for nn_DeformableUpdatingModel.

Math (same collapse as v1):
  out[m,o] = (1/HW) * ( sum_q wsum_m[q] * Fp[q,o] + be2[o] * s_m ) + b_dc[o]
  Fp = F^T Wc^T (combined 1x1 convs), wsum_m = scatter of bilinear weights,
  s_m = sum_q wsum_m[q].

v2 vs v1:
  - fp8e4 DoubleRow matmuls for Fp production (fk fp8 via casting DMA) and the
    final contraction (4x PE throughput each).
  - PSUM->SBUF crossing of Fp in (128,512) bank copies, fp8, split ACT/Pool.
  - Tents k-last so the subtract TensorTensor hits DVE 2x; T-side 16-wide
    skewed windows (free column offsets) as the moving operand; U-side full
    width stationary; wsum PSUM accumulates start=False onto a memset with
    u duplicated via two matmuls (bases 0/64) so scatters stay in-partition.
  - Flow transpose via constant selection matrices folded with 0.0625.
"""
import sys
if '/opt/trn_rl_repo' not in sys.path:
    sys.path.insert(0, '/opt/trn_rl_repo')

import numpy as np

import concourse.bacc as bacc
import concourse.mybir as mybir
import concourse.tile as tile
from concourse.bass_utils import run_bass_kernel_spmd

F32 = mybir.dt.float32
BF16 = mybir.dt.bfloat16
FP16 = mybir.dt.float16
FP8 = mybir.dt.float8e4
I32 = mybir.dt.int32
U16 = mybir.dt.uint16
OP = mybir.AluOpType
ACT = mybir.ActivationFunctionType
DR = mybir.MatmulPerfMode.DoubleRow

B, T, GOP = 4, 16, 4
NUM_GOP = T // GOP
NFLOW = 48
C = 256
H = W = 64
HW = H * W
NCORES = 8
GOPS_PER_CORE = 2
FLOWS_PER_CORE = 6

TW = 12          # T-side skewed window width; window for k is [2k-5, 2k+7)
VOFF = [2 * k - 5 for k in range(32)]


def build_nc():
    nc = bacc.Bacc("TRN2", target_bir_lowering=False, debug=False,
                   num_devices=NCORES)

    d_if = nc.dram_tensor("ifeat", [GOPS_PER_CORE, C, HW], F32, kind="ExternalInput")
    d_pm = nc.dram_tensor("pmot", [FLOWS_PER_CORE, 2, 256, 256], F32, kind="ExternalInput")
    d_wc = nc.dram_tensor("wc", [128, 2, C], F32, kind="ExternalInput")
    d_be2 = nc.dram_tensor("be2", [3, C], F32, kind="ExternalInput")
    d_bdc3 = nc.dram_tensor("bdc3", [3, C], F32, kind="ExternalInput")
    d_out = nc.dram_tensor("out", [FLOWS_PER_CORE, C], F32, kind="ExternalOutput")

    with tile.TileContext(nc) as tc:
        with (
            tc.tile_pool(name="const", bufs=1) as cpool,
            tc.tile_pool(name="wpool", bufs=1) as wpool,
            tc.tile_pool(name="fkp", bufs=2) as fkp,
            tc.tile_pool(name="fpt", bufs=2) as fptp,
            tc.tile_pool(name="work", bufs=3) as work,
            tc.tile_pool(name="flw", bufs=1) as flw,
            tc.tile_pool(name="tt", bufs=3) as ttp,
            tc.tile_pool(name="tu", bufs=3) as tup,
            tc.tile_pool(name="ws", bufs=2) as wsp,
            tc.tile_pool(name="ps_f", bufs=2, space="PSUM") as ps_f,
            tc.tile_pool(name="ps_w", bufs=2, space="PSUM") as ps_w,
            tc.tile_pool(name="ps_q", bufs=1, space="PSUM") as ps_q,
            tc.tile_pool(name="ps_o", bufs=1, space="PSUM") as ps_o,
        ):
            # ------------- input DMAs first -------------
            pts = []
            def load_pt(fg):
                pmv = d_pm[fg:fg + 1, :, :, :].squeeze(0) \
                    .rearrange("c (i f) w -> i c f w", f=4)
                pt = flw.tile([64, 2, 2, 256], F32, tag=f"pm{fg}", name=f"pt{fg}")
                nc.sync.dma_start(pt[:], pmv[:, :, 1:3, :])
                pts.append(pt)
            for fg in range(3):
                load_pt(fg)

            wct = wpool.tile([128, 2, C], FP8)
            nc.gpsimd.dma_start(wct[:], d_wc[:])
            be2 = wpool.tile([3, C], F32)
            nc.sync.dma_start(be2[:], d_be2[:])
            bdc3 = wpool.tile([3, C], F32)
            nc.sync.dma_start(bdc3[:], d_bdc3[:])

            # ------------- constants -------------
            ones16 = cpool.tile([128, 1], FP16)
            nc.gpsimd.memset(ones16[:], 1.0)


            # T-side skewed iota: iotw[p, j, k] = j - 7 - p//64 (k-independent)
            iotw = cpool.tile([128, TW, 32], FP16)
            nc.gpsimd.iota(iotw[:], pattern=[[1, TW], [0, 32]], base=-5,
                           channel_multiplier=0,
                           allow_small_or_imprecise_dtypes=True)
            nc.vector.tensor_scalar(iotw[64:128, :, :], iotw[64:128, :, :], 1.0,
                                    None, op0=OP.subtract)

            # U-side full-width iota: iotu[p, u, k] = u - p%64 (k-independent)
            iotu = cpool.tile([128, 64, 32], FP16)
            nc.gpsimd.iota(iotu[:], pattern=[[1, 64], [0, 32]], base=0,
                           channel_multiplier=-1,
                           allow_small_or_imprecise_dtypes=True)
            nc.vector.tensor_scalar(iotu[64:128, :, :], iotu[64:128, :, :], 64.0,
                                    None, op0=OP.add)

            # E_par[r, par, k] = 0.0625 * (r == 2k + par)  (transpose+downscale)
            eseli = cpool.tile([64, 2, 32], I32)
            nc.gpsimd.iota(eseli[:], pattern=[[-1, 2], [-2, 32]], base=0,
                           channel_multiplier=1)
            esel = cpool.tile([64, 2, 32], BF16)
            nc.gpsimd.tensor_scalar(esel[:], eseli[:], 0, 0.0625,
                                    op0=OP.is_equal, op1=OP.mult)

            # fk casting DMAs (after iotas on the gpsimd queue)
            fk = [fkp.tile([128, 2, HW], FP8, tag="fk", name=f"fk{g}")
                  for g in range(2)]
            nc.gpsimd.dma_start(
                fk[0][:], d_if[0].rearrange("(kc p) q -> p kc q", p=128))
            for fg in range(3, 6):
                load_pt(fg)
            nc.gpsimd.dma_start(
                fk[1][:], d_if[1].rearrange("(kc p) q -> p kc q", p=128))

            # ------------- all six flows: downsample + transpose + yx -------------
            yxs = []
            for fg in range(6):
                pt = pts[fg]
                tA = work.tile([64, 2, 2, 64], F32, tag="tA", name=f"tA{fg}")
                nc.vector.tensor_tensor(out=tA[:], in0=pt[:, :, :, 1:254:4],
                                        in1=pt[:, :, :, 2:255:4], op=OP.add)
                ds2 = work.tile([64, 2, 64], BF16, tag="ds2", name=f"ds2{fg}")
                nc.vector.tensor_tensor(out=ds2[:], in0=tA[:, :, 0, :],
                                        in1=tA[:, :, 1, :], op=OP.add)
                # pq[64*par + s, 32*c + k] = 0.0625 * ds2[2k+par, c, s]
                pq = ps_q.tile([128, 64], F32, tag="pq", name=f"pq{fg}")
                for comp in range(2):
                    for par in range(2):
                        nc.tensor.matmul(
                            pq[64 * par:64 * (par + 1), 32 * comp:32 * (comp + 1)],
                            ds2[:, comp, :], esel[:, par, :],
                            start=True, stop=True)
                yx = flw.tile([128, 2, 32], FP16, tag=f"yx{fg}", name=f"yx{fg}")
                nc.scalar.copy(yx[:], pq[:].rearrange("p (a b) -> p a b", a=2))
                yxs.append(yx)

            # ------------- per gop -------------
            # crossing engine per (g, 4-chunk bank-pair): A=ACT V=DVE
            # (GPSIMD cannot access PSUM on hardware)
            XENG = {0: "AAVAAVAA", 1: "AAAAAAAA"}

            def produce_banks(g, fpt, b0, b1):
                for b in range(b0, b1):
                    psf = ps_f.tile([128, 1024], F32, tag="psf",
                                    name=f"psf{g}_{b}")
                    for h in range(4):
                        c = 4 * b + h
                        nc.tensor.matmul(
                            psf[:, 256 * h:256 * (h + 1)],
                            fk[g][:, :, 128 * c:128 * (c + 1)],
                            wct[:], start=True, stop=True, perf_mode=DR)
                    dst = fpt[:, 4 * b:4 * b + 4, :]
                    src = psf[:].rearrange("p (a b) -> p a b", a=4)
                    if XENG[g][b] == "A":
                        nc.scalar.copy(dst, src)
                    else:
                        nc.vector.tensor_copy(dst, src)

            fpts = []
            wsumts = []
            for g in range(GOPS_PER_CORE):
                fpt = fptp.tile([128, 32, C], FP16, tag="fpt", name=f"fpt{g}")
                wsumt = wsp.tile([128, 32, 3], FP16, tag="wsumt", name=f"ws{g}")
                fpts.append(fpt)
                wsumts.append(wsumt)

                for mm in range(3):
                    fg = 3 * g + mm
                    yx = yxs[fg]
                    # T tents (skewed, negated): tT = min(|iotw - dy|, 1) - 1
                    teng = nc.gpsimd if fg % 2 == 1 else nc.vector
                    dT = ttp.tile([128, TW, 32], FP16, tag="dt", name=f"dt{fg}")
                    teng.tensor_tensor(
                        out=dT[:], in0=iotw[:],
                        in1=yx[:, 0:1, :].broadcast_to([128, TW, 32]),
                        op=OP.subtract)
                    nc.vector.tensor_scalar(dT[:].bitcast(U16), dT[:].bitcast(U16),
                                            0x7FFF, None, op0=OP.bitwise_and)
                    tT = ttp.tile([128, TW, 32], FP16, tag="tt", name=f"tt{fg}")
                    nc.vector.tensor_scalar(tT[:], dT[:], 1.0, 1.0,
                                            op0=OP.min, op1=OP.subtract)
                    # U tents: mU = min(|iotu - dx|, 1), 2 fused ops.
                    # (and + integer-min works: positive fp16 bit patterns
                    # order like unsigned ints; 0x3C00 is fp16 1.0)
                    tU = tup.tile([128, 64, 32], FP16, tag="tu", name=f"tu{fg}")
                    ueng = nc.gpsimd if fg >= 4 else nc.vector
                    ueng.tensor_tensor(
                        out=tU[:], in0=iotu[:],
                        in1=yx[:, 1:2, :].broadcast_to([128, 64, 32]),
                        op=OP.subtract)
                    nc.vector.tensor_scalar(tU[:].bitcast(U16), tU[:].bitcast(U16),
                                            0x7FFF, None, op0=OP.bitwise_and)
                    nc.vector.tensor_scalar(tU[:], tU[:], 1.0, 1.0,
                                            op0=OP.min, op1=OP.subtract)

                    # wsum: pw[u(+64dup), v] accumulated over k windows
                    pw = ps_w.tile([128, 64], F32, tag="pw", name=f"pw{fg}")
                    nc.vector.memset(pw[:], 0.0)
                    for k in range(32):
                        j0 = max(0, -VOFF[k])
                        j1 = min(TW, 64 - VOFF[k])
                        va, vb = VOFF[k] + j0, VOFF[k] + j1
                        last = (k == 31)
                        nc.tensor.matmul(pw[0:64, va:vb], tU[:, :, k],
                                         tT[:, j0:j1, k], start=False,
                                         stop=False, skip_group_check=True)
                        nc.tensor.matmul(pw[64:128, va:vb], tU[:, :, k],
                                         tT[:, j0:j1, k], start=False,
                                         stop=last, skip_group_check=True)
                    # scatter: wsumt[p, c, m] = wsum[u=p%64, v=2c+p//64]
                    seng = nc.vector if g == 1 else nc.scalar
                    if g == 1:
                        nc.vector.tensor_copy(wsumt[0:64, :, mm:mm + 1],
                                              pw[0:64, 0:64:2].unsqueeze(2))
                        nc.vector.tensor_copy(wsumt[64:128, :, mm:mm + 1],
                                              pw[64:128, 1:64:2].unsqueeze(2))
                    else:
                        nc.scalar.copy(wsumt[0:64, :, mm:mm + 1],
                                       pw[0:64, 0:64:2].unsqueeze(2))
                        nc.scalar.copy(wsumt[64:128, :, mm:mm + 1],
                                       pw[64:128, 1:64:2].unsqueeze(2))

                produce_banks(g, fpt, 0, 8)

                # --- contraction ---
                po = ps_o.tile([3, 272], F32, tag="po", name=f"po{g}")
                for c in range(32):
                    nc.tensor.matmul(po[:, 0:256], wsumt[:, c, :],
                                     fpt[:, c, :],
                                     start=(c == 0), stop=(c == 31),
                                     skip_group_check=True)
                    nc.tensor.matmul(po[:, 256:257], wsumt[:, c, :],
                                     ones16[:], start=(c == 0), stop=(c == 31),
                                     skip_group_check=True)

                aux = work.tile([3, C], F32, tag="aux", name=f"aux{g}")
                nc.vector.tensor_scalar(aux[:], be2[:], po[:, 256:257], None,
                                        op0=OP.mult)
                nc.vector.scalar_tensor_tensor(aux[:], in0=po[:, 0:256],
                                               scalar=1.0 / 256.0, in1=aux[:],
                                               op0=OP.mult, op1=OP.add)
                osb = work.tile([3, C], F32, tag="osb", name=f"osb{g}")
                nc.vector.scalar_tensor_tensor(osb[:], in0=aux[:],
                                               scalar=1.0 / HW, in1=bdc3[:],
                                               op0=OP.mult, op1=OP.add)
                nc.sync.dma_start(d_out[3 * g:3 * (g + 1), :], osb[:])

    nc.compile()
    return nc


_NC_CACHE = {}


def _get_nc():
    if "nc" not in _NC_CACHE:
        _NC_CACHE["nc"] = build_nc()
    return _NC_CACHE["nc"]


def make_in_maps(i_features, p_motions, W_emb, b_emb, W_dc, b_dc):
    i_features = np.ascontiguousarray(i_features, np.float32).reshape(16, C, HW)
    pm = np.ascontiguousarray(p_motions, np.float32).reshape(NFLOW, 2, 256, 256)
    wc = (np.asarray(W_dc, np.float64) @ np.asarray(W_emb, np.float64)) * 256.0
    # wct[p, kc, o] = 256 * Wc[o, 128kc + p]
    wcT = np.ascontiguousarray(
        wc.T.reshape(2, 128, C).transpose(1, 0, 2).astype(np.float32))
    be2 = (np.asarray(W_dc, np.float64) @ np.asarray(b_dc, np.float64) * 0
           + np.asarray(W_dc, np.float64) @ np.asarray(b_emb, np.float64))
    be2 = np.ascontiguousarray(
        np.repeat(be2.astype(np.float32)[None, :], 3, axis=0))
    bdc3 = np.ascontiguousarray(
        np.repeat(np.asarray(b_dc, np.float32)[None, :], 3, axis=0))
    in_maps = []
    for c in range(NCORES):
        in_maps.append({
            "ifeat": np.ascontiguousarray(i_features[2 * c:2 * c + 2]),
            "pmot": np.ascontiguousarray(pm[6 * c:6 * c + 6]),
            "wc": wcT,
            "be2": be2,
            "bdc3": bdc3,
        })
    return in_maps


def kernel(imgs, i_features, p_motions, W_emb, b_emb, W_dc, b_dc, _trace=False):
    nc = _get_nc()
    in_maps = make_in_maps(np.asarray(i_features), np.asarray(p_motions),
                           np.asarray(W_emb), np.asarray(b_emb),
                           np.asarray(W_dc), np.asarray(b_dc))
    res = run_bass_kernel_spmd(nc, in_maps, core_ids=list(range(NCORES)),
                               trace=_trace)
    out = np.concatenate([np.asarray(r["out"]) for r in res.results], axis=0)
    out = out.reshape(B, NUM_GOP, GOP - 1, C)
    if _trace:
        return out, res
    return out


# revision 3
# speedup vs baseline: 1.0057x; 1.0057x over previous
"""Trainium2 Bass kernel v2 for nn_DeformableUpdatingModel.

Math (same collapse as v1):
  out[m,o] = (1/HW) * ( sum_q wsum_m[q] * Fp[q,o] + be2[o] * s_m ) + b_dc[o]
  Fp = F^T Wc^T (combined 1x1 convs), wsum_m = scatter of bilinear weights,
  s_m = sum_q wsum_m[q].

v2 vs v1:
  - fp8e4 DoubleRow matmuls for Fp production (fk fp8 via casting DMA) and the
    final contraction (4x PE throughput each).
  - PSUM->SBUF crossing of Fp in (128,512) bank copies, fp8, split ACT/Pool.
  - Tents k-last so the subtract TensorTensor hits DVE 2x; T-side 16-wide
    skewed windows (free column offsets) as the moving operand; U-side full
    width stationary; wsum PSUM accumulates start=False onto a memset with
    u duplicated via two matmuls (bases 0/64) so scatters stay in-partition.
  - Flow transpose via constant selection matrices folded with 0.0625.
"""
import sys
if '/opt/trn_rl_repo' not in sys.path:
    sys.path.insert(0, '/opt/trn_rl_repo')

import numpy as np

import concourse.bacc as bacc
import concourse.mybir as mybir
import concourse.tile as tile
from concourse.bass_utils import run_bass_kernel_spmd

F32 = mybir.dt.float32
BF16 = mybir.dt.bfloat16
FP16 = mybir.dt.float16
FP8 = mybir.dt.float8e4
I32 = mybir.dt.int32
U16 = mybir.dt.uint16
OP = mybir.AluOpType
ACT = mybir.ActivationFunctionType
DR = mybir.MatmulPerfMode.DoubleRow

B, T, GOP = 4, 16, 4
NUM_GOP = T // GOP
NFLOW = 48
C = 256
H = W = 64
HW = H * W
NCORES = 8
GOPS_PER_CORE = 2
FLOWS_PER_CORE = 6

TW = 12          # T-side skewed window width; window for k is [2k-5, 2k+7)
VOFF = [2 * k - 5 for k in range(32)]


def build_nc():
    nc = bacc.Bacc("TRN2", target_bir_lowering=False, debug=False,
                   num_devices=NCORES)

    d_if = nc.dram_tensor("ifeat", [GOPS_PER_CORE, C, HW], F32, kind="ExternalInput")
    d_pm = nc.dram_tensor("pmot", [FLOWS_PER_CORE, 2, 256, 256], F32, kind="ExternalInput")
    d_wc = nc.dram_tensor("wc", [128, 2, C], F32, kind="ExternalInput")
    d_be2 = nc.dram_tensor("be2", [3, C], F32, kind="ExternalInput")
    d_bdc3 = nc.dram_tensor("bdc3", [3, C], F32, kind="ExternalInput")
    d_out = nc.dram_tensor("out", [FLOWS_PER_CORE, C], F32, kind="ExternalOutput")

    with tile.TileContext(nc) as tc:
        with (
            tc.tile_pool(name="const", bufs=1) as cpool,
            tc.tile_pool(name="wpool", bufs=1) as wpool,
            tc.tile_pool(name="fkp", bufs=2) as fkp,
            tc.tile_pool(name="fpt", bufs=2) as fptp,
            tc.tile_pool(name="work", bufs=3) as work,
            tc.tile_pool(name="flw", bufs=1) as flw,
            tc.tile_pool(name="tt", bufs=3) as ttp,
            tc.tile_pool(name="tu", bufs=3) as tup,
            tc.tile_pool(name="ws", bufs=2) as wsp,
            tc.tile_pool(name="ps_f", bufs=2, space="PSUM") as ps_f,
            tc.tile_pool(name="ps_w", bufs=2, space="PSUM") as ps_w,
            tc.tile_pool(name="ps_q", bufs=1, space="PSUM") as ps_q,
            tc.tile_pool(name="ps_o", bufs=1, space="PSUM") as ps_o,
        ):
            # ------------- input DMAs first -------------
            pts = []
            def load_pt(fg):
                pmv = d_pm[fg:fg + 1, :, :, :].squeeze(0) \
                    .rearrange("c (i f) w -> i c f w", f=4)
                pt = flw.tile([64, 2, 2, 256], F32, tag=f"pm{fg}", name=f"pt{fg}")
                nc.sync.dma_start(pt[:], pmv[:, :, 1:3, :])
                pts.append(pt)
            for fg in range(3):
                load_pt(fg)

            wct = wpool.tile([128, 2, C], FP8)
            nc.gpsimd.dma_start(wct[:], d_wc[:])
            be2 = wpool.tile([3, C], F32)
            nc.sync.dma_start(be2[:], d_be2[:])
            bdc3 = wpool.tile([3, C], F32)
            nc.sync.dma_start(bdc3[:], d_bdc3[:])

            # ------------- constants -------------
            ones16 = cpool.tile([128, 1], FP16)
            nc.gpsimd.memset(ones16[:], 1.0)


            # T-side skewed iota: iotw[p, j, k] = j - 7 - p//64 (k-independent)
            iotw = cpool.tile([128, TW, 32], FP16)
            nc.gpsimd.iota(iotw[:], pattern=[[1, TW], [0, 32]], base=-5,
                           channel_multiplier=0,
                           allow_small_or_imprecise_dtypes=True)
            nc.vector.tensor_scalar(iotw[64:128, :, :], iotw[64:128, :, :], 1.0,
                                    None, op0=OP.subtract)

            # U-side full-width iota: iotu[p, u, k] = u - p%64 (k-independent)
            iotu = cpool.tile([128, 64, 32], FP16)
            nc.gpsimd.iota(iotu[:], pattern=[[1, 64], [0, 32]], base=0,
                           channel_multiplier=-1,
                           allow_small_or_imprecise_dtypes=True)
            nc.vector.tensor_scalar(iotu[64:128, :, :], iotu[64:128, :, :], 64.0,
                                    None, op0=OP.add)

            # E_par[r, par, k] = 0.0625 * (r == 2k + par)  (transpose+downscale)
            eseli = cpool.tile([64, 2, 32], I32)
            nc.gpsimd.iota(eseli[:], pattern=[[-1, 2], [-2, 32]], base=0,
                           channel_multiplier=1)
            esel = cpool.tile([64, 2, 32], BF16)
            nc.gpsimd.tensor_scalar(esel[:], eseli[:], 0, 0.0625,
                                    op0=OP.is_equal, op1=OP.mult)

            # fk casting DMAs (after iotas on the gpsimd queue)
            fk = [fkp.tile([128, 2, HW], FP8, tag="fk", name=f"fk{g}")
                  for g in range(2)]
            nc.gpsimd.dma_start(
                fk[0][:, :, 0:2048],
                d_if[0].rearrange("(kc p) q -> p kc q", p=128)[:, :, 0:2048])
            nc.gpsimd.dma_start(
                fk[0][:, :, 2048:4096],
                d_if[0].rearrange("(kc p) q -> p kc q", p=128)[:, :, 2048:4096])
            for fg in range(3, 6):
                load_pt(fg)
            nc.gpsimd.dma_start(
                fk[1][:, :, 0:2048],
                d_if[1].rearrange("(kc p) q -> p kc q", p=128)[:, :, 0:2048])
            nc.gpsimd.dma_start(
                fk[1][:, :, 2048:4096],
                d_if[1].rearrange("(kc p) q -> p kc q", p=128)[:, :, 2048:4096])

            # ------------- all six flows: downsample + transpose + yx -------------
            yxs = []
            for fg in range(6):
                pt = pts[fg]
                tA = work.tile([64, 2, 2, 64], F32, tag="tA", name=f"tA{fg}")
                nc.vector.tensor_tensor(out=tA[:], in0=pt[:, :, :, 1:254:4],
                                        in1=pt[:, :, :, 2:255:4], op=OP.add)
                ds2 = work.tile([64, 2, 64], BF16, tag="ds2", name=f"ds2{fg}")
                nc.vector.tensor_tensor(out=ds2[:], in0=tA[:, :, 0, :],
                                        in1=tA[:, :, 1, :], op=OP.add)
                # pq[64*par + s, 32*c + k] = 0.0625 * ds2[2k+par, c, s]
                pq = ps_q.tile([128, 64], F32, tag="pq", name=f"pq{fg}")
                for comp in range(2):
                    for par in range(2):
                        nc.tensor.matmul(
                            pq[64 * par:64 * (par + 1), 32 * comp:32 * (comp + 1)],
                            ds2[:, comp, :], esel[:, par, :],
                            start=True, stop=True)
                yx = flw.tile([128, 2, 32], FP16, tag=f"yx{fg}", name=f"yx{fg}")
                nc.scalar.copy(yx[:], pq[:].rearrange("p (a b) -> p a b", a=2))
                yxs.append(yx)

            # ------------- per gop -------------
            # crossing engine per (g, 4-chunk bank-pair): A=ACT V=DVE
            # (GPSIMD cannot access PSUM on hardware)
            XENG = {0: "AAVAAVAA", 1: "AAAAAAAA"}

            def produce_banks(g, fpt, b0, b1):
                for b in range(b0, b1):
                    psf = ps_f.tile([128, 1024], F32, tag="psf",
                                    name=f"psf{g}_{b}")
                    for h in range(4):
                        c = 4 * b + h
                        nc.tensor.matmul(
                            psf[:, 256 * h:256 * (h + 1)],
                            fk[g][:, :, 128 * c:128 * (c + 1)],
                            wct[:], start=True, stop=True, perf_mode=DR)
                    dst = fpt[:, 4 * b:4 * b + 4, :]
                    src = psf[:].rearrange("p (a b) -> p a b", a=4)
                    if XENG[g][b] == "A":
                        nc.scalar.copy(dst, src)
                    else:
                        nc.vector.tensor_copy(dst, src)

            fpts = []
            wsumts = []
            for g in range(GOPS_PER_CORE):
                fpt = fptp.tile([128, 32, C], FP16, tag="fpt", name=f"fpt{g}")
                wsumt = wsp.tile([128, 32, 3], FP16, tag="wsumt", name=f"ws{g}")
                fpts.append(fpt)
                wsumts.append(wsumt)

                for mm in range(3):
                    fg = 3 * g + mm
                    yx = yxs[fg]
                    # T tents (skewed, negated): tT = min(|iotw - dy|, 1) - 1
                    teng = nc.gpsimd if fg % 2 == 1 else nc.vector
                    dT = ttp.tile([128, TW, 32], FP16, tag="dt", name=f"dt{fg}")
                    teng.tensor_tensor(
                        out=dT[:], in0=iotw[:],
                        in1=yx[:, 0:1, :].broadcast_to([128, TW, 32]),
                        op=OP.subtract)
                    nc.vector.tensor_scalar(dT[:].bitcast(U16), dT[:].bitcast(U16),
                                            0x7FFF, None, op0=OP.bitwise_and)
                    tT = ttp.tile([128, TW, 32], FP16, tag="tt", name=f"tt{fg}")
                    nc.vector.tensor_scalar(tT[:], dT[:], 1.0, 1.0,
                                            op0=OP.min, op1=OP.subtract)
                    # U tents: mU = min(|iotu - dx|, 1), 2 fused ops.
                    # (and + integer-min works: positive fp16 bit patterns
                    # order like unsigned ints; 0x3C00 is fp16 1.0)
                    tU = tup.tile([128, 64, 32], FP16, tag="tu", name=f"tu{fg}")
                    ueng = nc.gpsimd if fg >= 4 else nc.vector
                    ueng.tensor_tensor(
                        out=tU[:], in0=iotu[:],
                        in1=yx[:, 1:2, :].broadcast_to([128, 64, 32]),
                        op=OP.subtract)
                    nc.vector.tensor_scalar(tU[:].bitcast(U16), tU[:].bitcast(U16),
                                            0x7FFF, None, op0=OP.bitwise_and)
                    nc.vector.tensor_scalar(tU[:], tU[:], 1.0, 1.0,
                                            op0=OP.min, op1=OP.subtract)

                    # wsum: pw[u(+64dup), v] accumulated over k windows
                    pw = ps_w.tile([128, 64], F32, tag="pw", name=f"pw{fg}")
                    nc.vector.memset(pw[:], 0.0)
                    for k in range(32):
                        j0 = max(0, -VOFF[k])
                        j1 = min(TW, 64 - VOFF[k])
                        va, vb = VOFF[k] + j0, VOFF[k] + j1
                        last = (k == 31)
                        nc.tensor.matmul(pw[0:64, va:vb], tU[:, :, k],
                                         tT[:, j0:j1, k], start=False,
                                         stop=False, skip_group_check=True)
                        nc.tensor.matmul(pw[64:128, va:vb], tU[:, :, k],
                                         tT[:, j0:j1, k], start=False,
                                         stop=last, skip_group_check=True)
                    # scatter: wsumt[p, c, m] = wsum[u=p%64, v=2c+p//64]
                    seng = nc.vector if g == 1 else nc.scalar
                    if g == 1:
                        nc.vector.tensor_copy(wsumt[0:64, :, mm:mm + 1],
                                              pw[0:64, 0:64:2].unsqueeze(2))
                        nc.vector.tensor_copy(wsumt[64:128, :, mm:mm + 1],
                                              pw[64:128, 1:64:2].unsqueeze(2))
                    else:
                        nc.scalar.copy(wsumt[0:64, :, mm:mm + 1],
                                       pw[0:64, 0:64:2].unsqueeze(2))
                        nc.scalar.copy(wsumt[64:128, :, mm:mm + 1],
                                       pw[64:128, 1:64:2].unsqueeze(2))

                produce_banks(g, fpt, 0, 8)

                # --- contraction ---
                po = ps_o.tile([3, 272], F32, tag="po", name=f"po{g}")
                for c in range(32):
                    nc.tensor.matmul(po[:, 0:256], wsumt[:, c, :],
                                     fpt[:, c, :],
                                     start=(c == 0), stop=(c == 31),
                                     skip_group_check=True)
                    nc.tensor.matmul(po[:, 256:257], wsumt[:, c, :],
                                     ones16[:], start=(c == 0), stop=(c == 31),
                                     skip_group_check=True)

                aux = work.tile([3, C], F32, tag="aux", name=f"aux{g}")
                nc.vector.tensor_scalar(aux[:], be2[:], po[:, 256:257], None,
                                        op0=OP.mult)
                nc.vector.scalar_tensor_tensor(aux[:], in0=po[:, 0:256],
                                               scalar=1.0 / 256.0, in1=aux[:],
                                               op0=OP.mult, op1=OP.add)
                osb = work.tile([3, C], F32, tag="osb", name=f"osb{g}")
                nc.vector.scalar_tensor_tensor(osb[:], in0=aux[:],
                                               scalar=1.0 / HW, in1=bdc3[:],
                                               op0=OP.mult, op1=OP.add)
                nc.sync.dma_start(d_out[3 * g:3 * (g + 1), :], osb[:])

    nc.compile()
    return nc


_NC_CACHE = {}


def _get_nc():
    if "nc" not in _NC_CACHE:
        _NC_CACHE["nc"] = build_nc()
    return _NC_CACHE["nc"]


def make_in_maps(i_features, p_motions, W_emb, b_emb, W_dc, b_dc):
    i_features = np.ascontiguousarray(i_features, np.float32).reshape(16, C, HW)
    pm = np.ascontiguousarray(p_motions, np.float32).reshape(NFLOW, 2, 256, 256)
    wc = (np.asarray(W_dc, np.float64) @ np.asarray(W_emb, np.float64)) * 256.0
    # wct[p, kc, o] = 256 * Wc[o, 128kc + p]
    wcT = np.ascontiguousarray(
        wc.T.reshape(2, 128, C).transpose(1, 0, 2).astype(np.float32))
    be2 = (np.asarray(W_dc, np.float64) @ np.asarray(b_dc, np.float64) * 0
           + np.asarray(W_dc, np.float64) @ np.asarray(b_emb, np.float64))
    be2 = np.ascontiguousarray(
        np.repeat(be2.astype(np.float32)[None, :], 3, axis=0))
    bdc3 = np.ascontiguousarray(
        np.repeat(np.asarray(b_dc, np.float32)[None, :], 3, axis=0))
    in_maps = []
    for c in range(NCORES):
        in_maps.append({
            "ifeat": np.ascontiguousarray(i_features[2 * c:2 * c + 2]),
            "pmot": np.ascontiguousarray(pm[6 * c:6 * c + 6]),
            "wc": wcT,
            "be2": be2,
            "bdc3": bdc3,
        })
    return in_maps


def kernel(imgs, i_features, p_motions, W_emb, b_emb, W_dc, b_dc, _trace=False):
    nc = _get_nc()
    in_maps = make_in_maps(np.asarray(i_features), np.asarray(p_motions),
                           np.asarray(W_emb), np.asarray(b_emb),
                           np.asarray(W_dc), np.asarray(b_dc))
    res = run_bass_kernel_spmd(nc, in_maps, core_ids=list(range(NCORES)),
                               trace=_trace)
    out = np.concatenate([np.asarray(r["out"]) for r in res.results], axis=0)
    out = out.reshape(B, NUM_GOP, GOP - 1, C)
    if _trace:
        return out, res
    return out
